# revision 33
# baseline (speedup 1.0000x reference)
"""Decision Transformer on 8 Trainium2 NeuronCores.

Sharding: batch(4) x 2-way tensor parallel (head split for attention,
ff split for FFN). Core c: batch c//2, shard c%2. All cores run the same
instruction stream (SPMD); shard differences live entirely in the input
data. Two chunk-pipelined pair-AllReduces per layer.

On-chip layout: residual stream is kept transposed (x^T = [D, S]) so every
matmul reads it directly (as lhsT for row-layout outputs, as rhs for
transposed outputs) with no PE transposes. LayerNorm stats are computed with
ones-vector matmuls (column sums) and broadcast back with rank-1 matmuls.
Attention computes logits transposed ([k, q]), skips fully-masked causal
k-tiles, masks diagonal tiles with affine_select, and defers softmax
normalization until after probs@V via a ones-column appended to V.

All matmul operands are float32r (fp32 bits, full-rate PE mode).
"""

import numpy as np

import concourse.bass as bass
import concourse.mybir as mybir
import concourse.tile as tile
from concourse import bacc
from concourse.bass_utils import run_bass_kernel_spmd
from concourse.masks import make_identity

F32 = mybir.dt.float32
F32R = mybir.dt.float32r
BF16 = mybir.dt.bfloat16
I32 = mybir.dt.int32
AF = mybir.ActivationFunctionType
OP = mybir.AluOpType

N, L, D = 4, 512, 768
STATE, ACT_DIM = 17, 6
H, KD = 12, 64
FF = 2048
NL = 4
MAXT = 4096

S = 3 * L            # 1536 tokens
DT = D // 128        # 6 d-tiles
CW = 512             # chunk width (tokens)
NCH = S // CW        # 3 chunks
KT = S // 128        # 12 k-tiles
HD = H // 2          # 6 heads per core
HP = HD // 2         # 3 head pairs
FFH = FF // 2        # 1024 ff per core
FFC = FFH // 128     # 8 ff chunks
VW = HD * (KD + 1)   # 390: V layout with a ones column per head
EPS = 1e-5

REPLICA_GROUPS = [[0, 1], [2, 3], [4, 5], [6, 7]]


def build_nc():
    nc = bacc.Bacc("TRN2", target_bir_lowering=False, debug=False, num_devices=8)

    # ---- inputs (per core; host does the sharding) ----
    d_rT = nc.dram_tensor("rT", [2, L], F32R, kind="ExternalInput")
    d_sT = nc.dram_tensor("sT", [STATE + 1, L], F32R, kind="ExternalInput")
    d_aT = nc.dram_tensor("aT", [ACT_DIM + 1, L], F32R, kind="ExternalInput")
    d_tix = nc.dram_tensor("tix", [L, 1], I32, kind="ExternalInput")
    d_emb = nc.dram_tensor("emb", [MAXT, D], F32, kind="ExternalInput")
    d_wr = nc.dram_tensor("wr", [2, D], F32R, kind="ExternalInput")
    d_ws = nc.dram_tensor("ws", [STATE + 1, D], F32R, kind="ExternalInput")
    d_wa = nc.dram_tensor("wa", [ACT_DIM + 1, D], F32R, kind="ExternalInput")
    d_lng = nc.dram_tensor("lng", [9, D], F32, kind="ExternalInput")
    d_lnb = nc.dram_tensor("lnb", [9, D], F32, kind="ExternalInput")
    d_wq = nc.dram_tensor("wq", [NL, D, HD * KD], F32R, kind="ExternalInput")
    d_wk = nc.dram_tensor("wk", [NL, D, HD * KD], F32R, kind="ExternalInput")
    d_wv = nc.dram_tensor("wv", [NL, D, HD * KD], F32R, kind="ExternalInput")
    d_wo = nc.dram_tensor("wo", [NL, H * KD, D], BF16, kind="ExternalInput")
    d_w1 = nc.dram_tensor("w1", [NL, D, FF], F32R, kind="ExternalInput")
    d_b1 = nc.dram_tensor("b1", [NL, FF], F32, kind="ExternalInput")
    d_w2 = nc.dram_tensor("w2", [NL, FF, D], BF16, kind="ExternalInput")
    d_b2 = nc.dram_tensor("b2", [NL, D], F32, kind="ExternalInput")
    d_wpa = nc.dram_tensor("wpa", [D, ACT_DIM], F32R, kind="ExternalInput")
    d_bpa = nc.dram_tensor("bpa", [1, ACT_DIM], F32, kind="ExternalInput")
    d_out = nc.dram_tensor("outT", [ACT_DIM, L], F32, kind="ExternalOutput")

    with tile.TileContext(nc) as tc:
        with (
            tc.tile_pool(name="persist", bufs=1) as pp,
            tc.tile_pool(name="wq2", bufs=2) as wq2,       # wq/wk hpair blocks
            tc.tile_pool(name="wbig", bufs=1) as wbig,     # wv / wo per layer
            tc.tile_pool(name="wff", bufs=3) as wff,       # w1 / w2 chunks
            tc.tile_pool(name="act2", bufs=2) as act2,     # qTc / attnTc / pos
            tc.tile_pool(name="probs", bufs=3) as prp,
            tc.tile_pool(name="scr", bufs=3) as scr,       # [128, CW] scratch
            tc.tile_pool(name="rows", bufs=2) as rowsp,
            tc.tile_pool(name="small", bufs=3) as smallp,
            tc.tile_pool(name="ps", bufs=2, space="PSUM") as psA,
            tc.tile_pool(name="pslg", bufs=2, space="PSUM") as psLG,
            tc.tile_pool(name="pspv", bufs=2, space="PSUM") as psPV,
            tc.tile_pool(name="psmisc", bufs=2, space="PSUM") as psMISC,
            tc.tile_pool(name="dram", bufs=3, space="DRAM") as drp,
        ):
            # ---- persistent tiles ----
            x = pp.tile([128, DT, S], F32R)          # residual stream, transposed
            kT = pp.tile([128, HP, S], F32R)         # K^T (own heads)
            v = pp.tile([128, KT, VW], F32R)         # V rows + ones col per head
            lng_sb = pp.tile([128, 9, DT], F32)
            lnb_sb = pp.tile([128, 9, DT], F32)
            ident = pp.tile([128, 128], F32)
            ones_col = pp.tile([128, 1], F32R)
            onesP = pp.tile([1, 128], F32R)
            b1_sb = pp.tile([128, NL, 2 * FFC], F32)
            b2_sb = pp.tile([128, NL, DT], F32)
            bpa_sb = pp.tile([ACT_DIM, 1], F32)
            wpa_sb = pp.tile([128, DT, ACT_DIM], F32R)
            wr_sb = pp.tile([2, D], F32R)
            ws_sb = pp.tile([STATE + 1, D], F32R)
            wa_sb = pp.tile([ACT_DIM + 1, D], F32R)
            rT_sb = pp.tile([2, L], F32R)
            sT_sb = pp.tile([STATE + 1, L], F32R)
            aT_sb = pp.tile([ACT_DIM + 1, L], F32R)

            make_identity(nc, ident)
            ones_f = pp.tile([128, 128], F32)
            nc.vector.memset(ones_f, 1.0)
            nc.scalar.copy(out=ones_col, in_=ones_f[:, 0:1])
            nc.scalar.copy(out=onesP, in_=ones_f[0:1, :])
            for kt in range(KT):
                for hd in range(HD):
                    nc.scalar.copy(out=v[:, kt, hd * 65 + 64 : hd * 65 + 65], in_=ones_f[:, 0:1])

            nc.sync.dma_start(out=lng_sb, in_=d_lng.ap().rearrange("g (t p) -> p g t", p=128))
            nc.sync.dma_start(out=lnb_sb, in_=d_lnb.ap().rearrange("g (t p) -> p g t", p=128))
            nc.sync.dma_start(out=b1_sb, in_=d_b1.ap().rearrange("l (t p) -> p l t", p=128))
            nc.sync.dma_start(out=b2_sb, in_=d_b2.ap().rearrange("l (t p) -> p l t", p=128))
            nc.sync.dma_start(out=bpa_sb, in_=d_bpa.ap().rearrange("o c -> c o"))
            nc.sync.dma_start(out=wpa_sb, in_=d_wpa.ap().rearrange("(t p) c -> p t c", p=128))
            nc.sync.dma_start(out=wr_sb, in_=d_wr.ap())
            nc.sync.dma_start(out=ws_sb, in_=d_ws.ap())
            nc.sync.dma_start(out=wa_sb, in_=d_wa.ap())
            nc.sync.dma_start(out=rT_sb, in_=d_rT.ap())
            nc.sync.dma_start(out=sT_sb, in_=d_sT.ap())
            nc.sync.dma_start(out=aT_sb, in_=d_aT.ap())

            def x_kind(dt, kind):
                # token columns 3j+kind of x[:, dt, :] as [128, L]
                return x[:, dt, :].rearrange("p (j k) -> p k j", k=3)[:, kind, :]

            def xcols(c):
                cs = slice(c * CW, (c + 1) * CW)
                return lambda dt: x[:, dt, cs]

            def x_state(dt):
                return x[:, dt, :].rearrange("p (j k) -> p k j", k=3)[:, 1, :]

            # ---- layernorm (in place on columns cols(dt) of x), g index gi ----
            def layer_norm(gi, cols):
                ps_m = psMISC.tile([1, CW], F32, tag="stat")
                ps_s = psMISC.tile([1, CW], F32, tag="stat")
                for dt in range(DT):
                    sq = scr.tile([128, CW], F32R, tag="scr")
                    nc.scalar.activation(out=sq, in_=cols(dt).bitcast(F32), func=AF.Square)
                    nc.tensor.matmul(ps_m, lhsT=ones_col, rhs=cols(dt),
                                     start=(dt == 0), stop=(dt == DT - 1))
                    nc.tensor.matmul(ps_s, lhsT=ones_col, rhs=sq,
                                     start=(dt == 0), stop=(dt == DT - 1))
                mrow = rowsp.tile([1, CW], F32R, tag="rowr")
                nc.vector.tensor_scalar(out=mrow, in0=ps_m, scalar1=1.0 / D,
                                        scalar2=None, op0=OP.mult)
                m2 = rowsp.tile([1, CW], F32, tag="rowf")
                nc.vector.tensor_tensor(out=m2, in0=mrow.bitcast(F32), in1=mrow.bitcast(F32), op=OP.mult)
                ve = rowsp.tile([1, CW], F32, tag="rowf")
                nc.vector.tensor_scalar(out=ve, in0=ps_s, scalar1=1.0 / D,
                                        scalar2=EPS, op0=OP.mult, op1=OP.add)
                nc.vector.tensor_tensor(out=ve, in0=ve, in1=m2, op=OP.subtract)
                sr = rowsp.tile([1, CW], F32, tag="rowf")
                nc.scalar.activation(out=sr, in_=ve, func=AF.Sqrt)
                rstd_f = rowsp.tile([1, CW], F32, tag="rowf")
                nc.vector.reciprocal(out=rstd_f, in_=sr)
                rstd = rowsp.tile([1, CW], F32R, tag="rowr")
                nc.scalar.copy(out=rstd, in_=rstd_f)

                mb = psMISC.tile([128, CW], F32, tag="stat")
                nc.tensor.matmul(mb, lhsT=onesP, rhs=mrow, start=True, stop=True)
                rb = psMISC.tile([128, CW], F32, tag="stat")
                nc.tensor.matmul(rb, lhsT=onesP, rhs=rstd, start=True, stop=True)
                for dt in range(DT):
                    tmp = scr.tile([128, CW], F32, tag="scr")
                    nc.vector.tensor_tensor(out=tmp, in0=cols(dt).bitcast(F32), in1=mb, op=OP.subtract)
                    nc.vector.tensor_tensor(out=tmp, in0=tmp, in1=rb, op=OP.mult)
                    nc.vector.tensor_scalar(out=cols(dt), in0=tmp,
                                            scalar1=lng_sb[:, gi, dt : dt + 1],
                                            scalar2=lnb_sb[:, gi, dt : dt + 1],
                                            op0=OP.mult, op1=OP.add)

            # ---- embedding ----
            for r in range(L // 128):
                tix_sb = smallp.tile([128, 1], I32, tag="tix")
                nc.sync.dma_start(out=tix_sb, in_=d_tix.ap()[r * 128 : (r + 1) * 128, :])
                pos = act2.tile([128, D], F32, tag="pos", bufs=1)
                nc.gpsimd.indirect_dma_start(
                    out=pos, out_offset=None, in_=d_emb.ap(),
                    in_offset=bass.IndirectOffsetOnAxis(ap=tix_sb[:, :1], axis=0),
                )
                for dt in range(DT):
                    tp = psA.tile([128, 128], F32, tag="mm")
                    nc.tensor.transpose(out=tp, in_=pos[:, dt * 128 : (dt + 1) * 128], identity=ident)
                    for kind in range(3):
                        nc.scalar.copy(
                            out=x_kind(dt, kind)[:, r * 128 : (r + 1) * 128],
                            in_=tp,
                        )
            for dt in range(DT):
                for w_sb, t_sb, kind in ((wr_sb, rT_sb, 0), (ws_sb, sT_sb, 1), (wa_sb, aT_sb, 2)):
                    pe = psA.tile([128, L], F32, tag="mm")
                    nc.tensor.matmul(pe, lhsT=w_sb[:, dt * 128 : (dt + 1) * 128], rhs=t_sb,
                                     start=True, stop=True)
                    xk = x_kind(dt, kind)
                    nc.vector.tensor_tensor(out=xk, in0=xk.bitcast(F32), in1=pe, op=OP.add)
            for c in range(NCH):
                layer_norm(0, xcols(c))

            # full FFN computed redundantly on both pair cores (no AllReduce):
            # FF split into two halves of 8 ff-tiles so only 8 h tiles live at once
            def ffn_chunk(li, cols):
                yas = []
                for half in range(2):
                    hts = []
                    for ffc in range(FFC):
                        gffc = half * FFC + ffc
                        w1_t = wff.tile([128, DT, 128], F32R, tag="w1")
                        nc.sync.dma_start(
                            out=w1_t,
                            in_=d_w1.ap()[li].rearrange("(t p) c -> p t c", p=128)[:, :, gffc * 128 : (gffc + 1) * 128],
                        )
                        ph = psA.tile([128, CW], F32, tag="mm")
                        for dt in range(DT):
                            nc.tensor.matmul(ph, lhsT=w1_t[:, dt, :], rhs=cols(dt),
                                             start=(dt == 0), stop=(dt == DT - 1))
                        ht = smallp.tile([128, CW], BF16, tag="hT", bufs=8, name=f"ht{ffc}")
                        nc.scalar.activation(out=ht, in_=ph, func=AF.Relu,
                                             bias=b1_sb[:, li, gffc : gffc + 1], scale=1.0)
                        hts.append(ht)
                    for dc in range(DT):
                        w2_t = wff.tile([128, FFC, 128], BF16, tag="w2", bufs=3)
                        nc.sync.dma_start(
                            out=w2_t,
                            in_=d_w2.ap()[li][half * FFH : (half + 1) * FFH, dc * 128 : (dc + 1) * 128]
                                .rearrange("(t p) c -> p t c", p=128),
                        )
                        ps_y = psA.tile([128, CW], F32, tag="mm")
                        for ffc in range(FFC):
                            nc.tensor.matmul(ps_y, lhsT=w2_t[:, ffc, :], rhs=hts[ffc],
                                             start=(ffc == 0), stop=(ffc == FFC - 1))
                        if half == 0:
                            ya = smallp.tile([128, CW], BF16, tag="ya", bufs=7, name=f"ya{dc}")
                            nc.scalar.copy(out=ya, in_=ps_y)
                            yas.append(ya)
                        else:
                            t1 = scr.tile([128, CW], F32, tag="scr")
                            nc.scalar.activation(out=t1, in_=ps_y, func=AF.Identity,
                                                 bias=b2_sb[:, li, dc : dc + 1], scale=1.0)
                            nc.vector.tensor_tensor(out=t1, in0=t1, in1=yas[dc], op=OP.add)
                            nc.vector.tensor_tensor(out=cols(dc), in0=cols(dc).bitcast(F32),
                                                    in1=t1, op=OP.add)

            # ---- transformer layers (all but last) ----
            for li in range(NL - 1):
                # K^T and V for all tokens
                wv_sb = wbig.tile([128, DT, HD * KD], F32R, tag="wv")
                nc.sync.dma_start(out=wv_sb, in_=d_wv.ap()[li].rearrange("(t p) c -> p t c", p=128))
                wk_sb = wq2.tile([128, DT, HD * KD], F32R, tag="wk", bufs=1)
                nc.sync.dma_start(out=wk_sb, in_=d_wk.ap()[li].rearrange("(t p) c -> p t c", p=128))
                for c in range(NCH):
                    cs = slice(c * CW, (c + 1) * CW)
                    for hp in range(HP):
                        pk = psA.tile([128, CW], F32, tag="mm")
                        for dt in range(DT):
                            nc.tensor.matmul(pk, lhsT=wk_sb[:, dt, hp * 128 : (hp + 1) * 128], rhs=x[:, dt, cs],
                                             start=(dt == 0), stop=(dt == DT - 1))
                        nc.vector.tensor_copy(out=kT[:, hp, cs], in_=pk)
                for kt in range(KT):
                    pv_ = psA.tile([128, HD * KD], F32, tag="mm")
                    for dt in range(DT):
                        nc.tensor.matmul(pv_, lhsT=x[:, dt, kt * 128 : (kt + 1) * 128], rhs=wv_sb[:, dt, :],
                                         start=(dt == 0), stop=(dt == DT - 1))
                    nc.scalar.copy(
                        out=v[:, kt, :].rearrange("p (h w) -> p h w", w=65)[:, :, 0:64],
                        in_=pv_.rearrange("p (h w) -> p h w", w=64),
                    )

                wq_sb = wq2.tile([128, DT, HD * KD], F32R, tag="wq", bufs=1)
                nc.sync.dma_start(out=wq_sb, in_=d_wq.ap()[li].rearrange("(t p) c -> p t c", p=128))
                wo_sb = wbig.tile([128, 2 * HP, D], BF16, tag="wo")
                nc.sync.dma_start(out=wo_sb, in_=d_wo.ap()[li].rearrange("(t p) c -> p t c", p=128))

                ar1_out = [None] * NCH
                def attn_chunk(c, li=li, wq_sb=wq_sb, wo_sb=wo_sb, ar1_out=ar1_out):
                    cs = slice(c * CW, (c + 1) * CW)
                    qTc = act2.tile([128, HP, CW], F32R, tag="qTc")
                    for hp in range(HP):
                        pq = psA.tile([128, CW], F32, tag="mm")
                        for dt in range(DT):
                            nc.tensor.matmul(pq, lhsT=wq_sb[:, dt, hp * 128 : (hp + 1) * 128], rhs=x[:, dt, cs],
                                             start=(dt == 0), stop=(dt == DT - 1))
                        nc.vector.tensor_copy(out=qTc[:, hp, :], in_=pq)
                    attnTc = act2.tile([128, HP, CW], BF16, tag="attnTc", bufs=2)
                    nkt = 4 * (c + 1)
                    for hd in range(HD):
                        hp, hi = hd // 2, hd % 2
                        prow = slice(64 * hi, 64 * hi + 64)
                        pv = psPV.tile([65, CW], F32, tag="pv")
                        for kt in range(nkt):
                            lg = psLG.tile([128, CW], F32, tag="lg")
                            nc.tensor.matmul(lg, lhsT=kT[prow, hp, kt * 128 : (kt + 1) * 128],
                                             rhs=qTc[prow, hp, :], start=True, stop=True)
                            pr = prp.tile([128, CW], F32R, tag="pr")
                            nc.scalar.activation(out=pr, in_=lg, func=AF.Exp, scale=float(KD) ** -0.5)
                            if kt >= 4 * c:
                                nc.gpsimd.affine_select(
                                    out=pr, in_=pr, compare_op=OP.is_ge, fill=0.0,
                                    base=c * CW - kt * 128, channel_multiplier=-1,
                                    pattern=[[1, CW]],
                                )
                            nc.tensor.matmul(pv, lhsT=v[:, kt, hd * 65 : hd * 65 + 65], rhs=pr,
                                             start=(kt == 0), stop=(kt == nkt - 1))
                        nc.scalar.copy(out=attnTc[prow, hp, :], in_=pv[0:64, :])
                        rc_f = rowsp.tile([1, CW], F32, tag="rowf")
                        nc.vector.reciprocal(out=rc_f, in_=pv[64:65, :])
                        rc = rowsp.tile([1, CW], F32R, tag="rowr")
                        nc.scalar.copy(out=rc, in_=rc_f)
                        bc = psMISC.tile([64, CW], F32, tag="stat")
                        nc.tensor.matmul(bc, lhsT=onesP[:, 0:64], rhs=rc,
                                         start=True, stop=True)
                        nc.vector.tensor_tensor(out=attnTc[prow, hp, :],
                                                in0=attnTc[prow, hp, :],
                                                in1=bc, op=OP.mult)
                    # AllGather own-head attn^T; full Wo is applied after the gather
                    ag_in = drp.tile([HP * 128, CW], BF16, tag="arin1")
                    ag_o = drp.tile([2 * HP * 128, CW], BF16, tag="arout1")
                    nc.sync.dma_start(out=ag_in.rearrange("(t p) c -> p t c", p=128), in_=attnTc)
                    nc.gpsimd.collective_compute(
                        "AllGather", OP.bypass, replica_groups=REPLICA_GROUPS,
                        ins=[ag_in.opt()], outs=[ag_o.opt()],
                    )
                    ar1_out[c] = ag_o

                def ln1_ffn_chunk(c, li=li, wo_sb=wo_sb):
                    cs = slice(c * CW, (c + 1) * CW)
                    gat = act2.tile([128, 2 * HP, CW], BF16, tag="gat", bufs=1)
                    nc.sync.dma_start(out=gat, in_=ar1_out[c].rearrange("(t p) c -> p t c", p=128))
                    for dc in range(DT):
                        py = psA.tile([128, CW], F32, tag="mm")
                        for hv in range(2 * HP):
                            nc.tensor.matmul(py, lhsT=wo_sb[:, hv, dc * 128 : (dc + 1) * 128],
                                             rhs=gat[:, hv, :],
                                             start=(hv == 0), stop=(hv == 2 * HP - 1))
                        nc.vector.tensor_tensor(out=x[:, dc, cs], in0=x[:, dc, cs].bitcast(F32),
                                                in1=py, op=OP.add)
                    layer_norm(1 + li, xcols(c))
                    ffn_chunk(li, lambda dt: x[:, dt, cs])
                    layer_norm(5 + li, xcols(c))

                # emission order chosen to keep the collective engine saturated
                attn_chunk(0)
                attn_chunk(1)
                ln1_ffn_chunk(0)
                attn_chunk(2)
                ln1_ffn_chunk(1)
                ln1_ffn_chunk(2)

            # ---- last layer: only state-token queries matter downstream ----
            li = NL - 1
            wv_sb = wbig.tile([128, DT, HD * KD], F32R, tag="wv")
            nc.sync.dma_start(out=wv_sb, in_=d_wv.ap()[li].rearrange("(t p) c -> p t c", p=128))
            wk_sb = wq2.tile([128, DT, HD * KD], F32R, tag="wk", bufs=1)
            nc.sync.dma_start(out=wk_sb, in_=d_wk.ap()[li].rearrange("(t p) c -> p t c", p=128))
            for c in range(NCH):
                cs = slice(c * CW, (c + 1) * CW)
                for hp in range(HP):
                    pk = psA.tile([128, CW], F32, tag="mm")
                    for dt in range(DT):
                        nc.tensor.matmul(pk, lhsT=wk_sb[:, dt, hp * 128 : (hp + 1) * 128], rhs=x[:, dt, cs],
                                         start=(dt == 0), stop=(dt == DT - 1))
                    nc.vector.tensor_copy(out=kT[:, hp, cs], in_=pk)
            for kt in range(KT):
                pv_ = psA.tile([128, HD * KD], F32, tag="mm")
                for dt in range(DT):
                    nc.tensor.matmul(pv_, lhsT=x[:, dt, kt * 128 : (kt + 1) * 128], rhs=wv_sb[:, dt, :],
                                     start=(dt == 0), stop=(dt == DT - 1))
                nc.scalar.copy(
                    out=v[:, kt, :].rearrange("p (h w) -> p h w", w=65)[:, :, 0:64],
                    in_=pv_.rearrange("p (h w) -> p h w", w=64),
                )
            wq_sb = wq2.tile([128, DT, HD * KD], F32R, tag="wq", bufs=1)
            nc.sync.dma_start(out=wq_sb, in_=d_wq.ap()[li].rearrange("(t p) c -> p t c", p=128))
            wo_sb = wbig.tile([128, 2 * HP, D], BF16, tag="wo")
            nc.sync.dma_start(out=wo_sb, in_=d_wo.ap()[li].rearrange("(t p) c -> p t c", p=128))

            # state-query attention: q = tokens 3j+1
            qTs = act2.tile([128, HP, CW], F32R, tag="qTc")
            for hp in range(HP):
                pq = psA.tile([128, CW], F32, tag="mm")
                for dt in range(DT):
                    nc.tensor.matmul(pq, lhsT=wq_sb[:, dt, hp * 128 : (hp + 1) * 128], rhs=x_state(dt),
                                     start=(dt == 0), stop=(dt == DT - 1))
                nc.vector.tensor_copy(out=qTs[:, hp, :], in_=pq)
            attnTs = act2.tile([128, HP, CW], BF16, tag="attnTc", bufs=2)
            for hd in range(HD):
                hp, hi = hd // 2, hd % 2
                prow = slice(64 * hi, 64 * hi + 64)
                pv = psPV.tile([65, CW], F32, tag="pv")
                for kt in range(KT):
                    lg = psLG.tile([128, CW], F32, tag="lg")
                    nc.tensor.matmul(lg, lhsT=kT[prow, hp, kt * 128 : (kt + 1) * 128],
                                     rhs=qTs[prow, hp, :], start=True, stop=True)
                    pr = prp.tile([128, CW], F32R, tag="pr")
                    nc.scalar.activation(out=pr, in_=lg, func=AF.Exp, scale=float(KD) ** -0.5)
                    nc.gpsimd.affine_select(
                        out=pr, in_=pr, compare_op=OP.is_ge, fill=0.0,
                        base=1 - kt * 128, channel_multiplier=-1,
                        pattern=[[3, CW]],
                    )
                    nc.tensor.matmul(pv, lhsT=v[:, kt, hd * 65 : hd * 65 + 65], rhs=pr,
                                     start=(kt == 0), stop=(kt == KT - 1))
                nc.scalar.copy(out=attnTs[prow, hp, :], in_=pv[0:64, :])
                rc_f = rowsp.tile([1, CW], F32, tag="rowf")
                nc.vector.reciprocal(out=rc_f, in_=pv[64:65, :])
                rc = rowsp.tile([1, CW], F32R, tag="rowr")
                nc.scalar.copy(out=rc, in_=rc_f)
                bc = psMISC.tile([64, CW], F32, tag="stat")
                nc.tensor.matmul(bc, lhsT=onesP[:, 0:64], rhs=rc, start=True, stop=True)
                nc.vector.tensor_tensor(out=attnTs[prow, hp, :],
                                        in0=attnTs[prow, hp, :],
                                        in1=bc, op=OP.mult)
            ag_in = drp.tile([HP * 128, CW], BF16, tag="arin1")
            ag_o = drp.tile([2 * HP * 128, CW], BF16, tag="arout1")
            nc.sync.dma_start(out=ag_in.rearrange("(t p) c -> p t c", p=128), in_=attnTs)
            nc.gpsimd.collective_compute(
                "AllGather", OP.bypass, replica_groups=REPLICA_GROUPS,
                ins=[ag_in.opt()], outs=[ag_o.opt()],
            )
            gat = act2.tile([128, 2 * HP, CW], BF16, tag="gat", bufs=1)
            nc.sync.dma_start(out=gat, in_=ag_o.rearrange("(t p) c -> p t c", p=128))
            for dc in range(DT):
                py = psA.tile([128, CW], F32, tag="mm")
                for hv in range(2 * HP):
                    nc.tensor.matmul(py, lhsT=wo_sb[:, hv, dc * 128 : (dc + 1) * 128],
                                     rhs=gat[:, hv, :],
                                     start=(hv == 0), stop=(hv == 2 * HP - 1))
                nc.vector.tensor_tensor(out=x_state(dc), in0=x_state(dc).bitcast(F32),
                                        in1=py, op=OP.add)
            layer_norm(1 + li, x_state)
            ffn_chunk(li, x_state)
            layer_norm(5 + li, x_state)

            # ---- output head on state tokens ----
            po = psA.tile([ACT_DIM, L], F32, tag="mm")
            for dt in range(DT):
                nc.tensor.matmul(po, lhsT=wpa_sb[:, dt, :],
                                 rhs=x[:, dt, :].rearrange("p (j k) -> p k j", k=3)[:, 1, :],
                                 start=(dt == 0), stop=(dt == DT - 1))
            ot = scr.tile([ACT_DIM, L], F32, tag="scr")
            nc.scalar.activation(out=ot, in_=po, func=AF.Identity, bias=bpa_sb, scale=1.0)
            nc.sync.dma_start(out=d_out.ap(), in_=ot)

    nc.compile()
    return nc


_NC_CACHE = None


def _get_nc():
    global _NC_CACHE
    if _NC_CACHE is None:
        _NC_CACHE = build_nc()
    return _NC_CACHE


def _make_in_maps(inputs):
    f32 = lambda a: np.ascontiguousarray(np.asarray(a, dtype=np.float32))
    R, s, a, t = f32(inputs["R"]), f32(inputs["s"]), f32(inputs["a"]), np.asarray(inputs["t"])
    ones = np.ones((1, L), np.float32)
    lng = np.concatenate([f32(inputs["ln0_g"])[None], f32(inputs["ln1_g"]), f32(inputs["ln2_g"])], 0)
    lnb = np.concatenate([f32(inputs["ln0_b"])[None], f32(inputs["ln1_b"]), f32(inputs["ln2_b"])], 0)
    wr = np.concatenate([f32(inputs["Wr"]), f32(inputs["br"])[None]], 0)
    ws = np.concatenate([f32(inputs["Ws"]), f32(inputs["bs"])[None]], 0)
    wa = np.concatenate([f32(inputs["Wa"]), f32(inputs["ba"])[None]], 0)
    emb = f32(inputs["embed_t"])
    Wq, Wk, Wv, Wo = f32(inputs["Wq"]), f32(inputs["Wk"]), f32(inputs["Wv"]), f32(inputs["Wo"])
    import ml_dtypes
    W1, b1, W2, b2 = f32(inputs["W1"]), f32(inputs["b1"]), f32(inputs["W2"]), f32(inputs["b2"])
    W2 = W2.astype(ml_dtypes.bfloat16)
    Wo_bf = np.ascontiguousarray(Wo.astype(ml_dtypes.bfloat16))
    wpa, bpa = f32(inputs["Wpa"]), f32(inputs["bpa"])

    in_maps = []
    for c in range(8):
        b, hh = c // 2, c % 2
        hs = slice(hh * HD * KD, (hh + 1) * HD * KD)
        fs = slice(hh * FFH, (hh + 1) * FFH)
        in_maps.append({
            "rT": np.ascontiguousarray(np.concatenate([R[b].T, ones], 0)),
            "sT": np.ascontiguousarray(np.concatenate([s[b].T, ones], 0)),
            "aT": np.ascontiguousarray(np.concatenate([a[b].T, ones], 0)),
            "tix": np.ascontiguousarray(t[b].astype(np.int32).reshape(L, 1)),
            "emb": emb,
            "wr": wr, "ws": ws, "wa": wa,
            "lng": lng, "lnb": lnb,
            "wq": np.ascontiguousarray(Wq[:, :, hs]),
            "wk": np.ascontiguousarray(Wk[:, :, hs]),
            "wv": np.ascontiguousarray(Wv[:, :, hs]),
            "wo": Wo_bf,
            "w1": W1,
            "b1": b1,
            "w2": W2,
            "b2": b2,
            "wpa": wpa,
            "bpa": bpa.reshape(1, ACT_DIM),
        })
    return in_maps


def run_on_device(inputs, trace=False):
    nc = _get_nc()
    in_maps = _make_in_maps(inputs)
    res = run_bass_kernel_spmd(nc, in_maps, core_ids=list(range(8)), trace=trace)
    out = np.stack([res.results[2 * b]["outT"].T for b in range(N)], 0)
    return out.astype(np.float32), res


def kernel(**inputs):
    try:
        out, _ = run_on_device(inputs, trace=False)
    except Exception:
        # transient device errors (e.g. NRT_EXEC_UNIT_UNRECOVERABLE) usually
        # clear on retry
        out, _ = run_on_device(inputs, trace=False)
    return out
